# revision 33
# baseline (speedup 1.0000x reference)
"""Trainium2 Bass kernel for Physics-Attention over an irregular mesh.

Contract: kernel(**inputs) takes the FULL inputs from setup_inputs() and
returns the FULL [4, 32768, 256] f32 output, distributing across 8 cores
internally (one (batch, half-of-N) shard per core, pairwise AllReduce on the
slice-token pooling reductions).

Structure (single core):
  pass 1 (per 128-token tile): logits/fx matmuls (bf16, chunked K=256),
    exp -> per-head denom -> reciprocal -> normalized routing weights swn;
    slice-token accumulation via pair-merged fp32-accumulating matmuls,
    software-pipelined ST_DELAY tiles behind the mains so the PE stays in
    long bursts (high p-state); swn is transposed for pass 2 by the DMA
    XBAR (sync queue), input loads ride the gpsimd software queue.
  stage: pairwise AllReduce of slice-token partials, tiny slice attention.
  pass 2: out = swT @ C in 8-tile supertiles, psum-bank-rotated matmuls,
    bf16 output written via batched DMA (host upcasts to f32).
"""

import sys

sys.path.insert(0, "/opt/trn_rl_repo")

import numpy as np
import ml_dtypes

import concourse.bass as bass
import concourse.mybir as mybir
import concourse.tile as tile
from concourse import bacc, bass_utils
from concourse.bass import ts

F32 = mybir.dt.float32
BF16 = mybir.dt.bfloat16
AF = mybir.ActivationFunctionType
ALU = mybir.AluOpType

B, N, DIM = 4, 32768, 256
H, D, G = 8, 64, 64
INNER = H * D  # 512
NCORES = 8
NLOC = N // 2          # 16384 tokens per core
TOK = 128              # tokens per tile
T = NLOC // TOK        # 128 tiles
EPS_SLICE = 1e-5
ST_DELAY = 4           # software-pipeline delay of the st matmuls (tiles)
SUP = 8                # pass-2 supertile (tiles per output DMA)

_CACHE = {}


def _build(attn_scale: float, res_scale: float, debug: bool = False):
    """Build the single-core SPMD program (identical on all 8 cores)."""
    nc = bacc.Bacc("TRN2", target_bir_lowering=False, debug=False,
                   enable_asserts=False, num_devices=NCORES)

    xT_d = nc.dram_tensor("xT", [128, 2, NLOC], BF16, kind="ExternalInput").ap()
    AT_d = nc.dram_tensor("AT", [128, 2, INNER], BF16, kind="ExternalInput").ap()
    WfxT_d = nc.dram_tensor("WfxT", [128, 2, INNER], BF16, kind="ExternalInput").ap()
    id32_d = nc.dram_tensor("id32", [64, 64], F32, kind="ExternalInput").ap()
    WqT_d = nc.dram_tensor("WqT", [D, D], F32, kind="ExternalInput").ap()
    WkT_d = nc.dram_tensor("WkT", [D, D], F32, kind="ExternalInput").ap()
    WvT_d = nc.dram_tensor("WvT", [D, D], F32, kind="ExternalInput").ap()
    WoT_d = nc.dram_tensor("WoT", [INNER, DIM], BF16, kind="ExternalInput").ap()
    outT_d = nc.dram_tensor("outT", [2, 128, NLOC], BF16, kind="ExternalOutput").ap()

    WoT_v = WoT_d.rearrange("(h d) f -> d h f", d=64)   # [64, 8, 256]
    # transposed output view: [p, fc, n] for one DMA per 4-tile group
    outT_v = outT_d.rearrange("fc p n -> p fc n")

    with tile.TileContext(nc) as tc:
        with (
            tc.tile_pool(name="consts", bufs=1) as consts,
            tc.tile_pool(name="store", bufs=1) as store,
            tc.tile_pool(name="work", bufs=3) as work,
            tc.tile_pool(name="uswp", bufs=8) as uswp,
            tc.tile_pool(name="small", bufs=6) as small,
            tc.tile_pool(name="stage", bufs=1) as stg_pool,
            tc.tile_pool(name="obuf", bufs=2) as obuf,
            tc.tile_pool(name="dram", bufs=1, space="DRAM") as dram,
        ):
            # ---- resident constants ----
            AT_sb = consts.tile([128, 2, INNER], BF16)
            nc.sync.dma_start(AT_sb, AT_d)
            WfxT_sb = consts.tile([128, 2, INNER], BF16)
            nc.sync.dma_start(WfxT_sb, WfxT_d)
            id32 = consts.tile([64, 64], F32)
            nc.sync.dma_start(id32, id32_d)
            WqT_sb = consts.tile([64, 64], F32)
            nc.sync.dma_start(WqT_sb, WqT_d)
            WkT_sb = consts.tile([64, 64], F32)
            nc.sync.dma_start(WkT_sb, WkT_d)
            WvT_sb = consts.tile([64, 64], F32)
            nc.sync.dma_start(WvT_sb, WvT_d)
            WoT_sb = consts.tile([64, H, DIM], BF16)
            nc.sync.dma_start(WoT_sb, WoT_v)

            # transposed routing weights, written by DMA transpose:
            # swT_store[p, t, c, j] = swn_t[j, c*128 + p]
            swT_store = store.tile([128, T, 4, TOK], BF16)

            # persistent swn pair tiles (4-deep manual rotation): pair P holds
            # tiles 2P, 2P+1; DMA-transposed together after the odd tile
            swn_tiles = [consts.tile([128, 2, H * G], BF16, name=f"swn{i}")
                         for i in range(4)]

            # persistent fxs tiles (6-deep manual rotation), ones columns
            # preset once: fxs*[p, c, half, 0:64] = fx data, [.., 64] = 1.
            # Split into a scalar-written set (pairs 0-1) and a vector-written
            # set (pairs 2-3) so the two engines share no tile (avoids false
            # WAW lockstep).
            fxsA_tiles = [consts.tile([128, 2, 2, D + 1], BF16, name=f"fxsA{i}")
                          for i in range(6)]
            fxsB_tiles = [consts.tile([128, 2, 2, D + 1], BF16, name=f"fxsB{i}")
                          for i in range(6)]
            for i in range(6):
                nc.gpsimd.memset(fxsA_tiles[i][:, :, :, D], 1.0)
                nc.gpsimd.memset(fxsB_tiles[i][:, :, :, D], 1.0)

            C_sb = stg_pool.tile([128, 4, DIM], BF16)

            with (
                tc.tile_pool(name="psmm", bufs=3, space="PSUM") as psmm,
                tc.tile_pool(name="psacc", bufs=1, space="PSUM") as psacc,
            ):
                # slice-token accumulators: pair c = heads (2c, 2c+1) lives in
                # st_ps[c//2][:, c%2, :]; valid regions: head 2c ->
                # [0:64, 0:65], head 2c+1 -> [64:128, 65:130].
                st_ps = [psacc.tile([128, 2, 2 * (D + 1)], F32, name=f"st_ps{j}")
                         for j in range(2)]

                # ================= PASS 1 =================
                # processed in tile PAIRS: PE does mains for both tiles
                # back-to-back, then the delayed st matmuls for an earlier
                # pair — long PE bursts keep the tensor engine at high pstate.
                # The elementwise chain (exp/reduce/recip/mult) is fused at
                # pair granularity to amortize fixed costs; fxs casts are
                # emitted after the chain so they can't block the next exp.
                hist = {}  # tile t -> (lg, fxp) psum handles

                def emit_chain(t):
                    lg, fxp = hist[t]
                    usw = uswp.tile([128, H, G], BF16, tag="usw")
                    nc.scalar.activation(usw, lg.rearrange("p (h g) -> p h g", h=H),
                                         AF.Exp)
                    den = small.tile([128, H], BF16, tag="den")
                    rden = small.tile([128, H], BF16, tag="rden")
                    with nc.allow_low_precision(reason="softmax denom in bf16"):
                        nc.vector.reduce_sum(den, usw, axis=mybir.AxisListType.X)
                        nc.vector.reciprocal(rden, den)
                    swn_pair = swn_tiles[(t // 2) % 4]
                    swn = swn_pair[:, t % 2].rearrange("p (h g) -> p h g", h=H)
                    nc.gpsimd.tensor_tensor(
                        swn, usw, rden[:, :, None].to_broadcast([128, H, G]),
                        ALU.mult)
                    if t % 2 == 1:
                        # transposed copy for pass 2, on the DMA engines
                        nc.sync.dma_start(
                            swT_store[:, t - 1:t + 1].rearrange(
                                "p a b j -> p (a b) j"),
                            swn_pair.rearrange("p a f -> p (a f)"),
                            transpose=True)

                def emit_fxs(t):
                    _, fxp = hist.pop(t)
                    fxsA = fxsA_tiles[t % 6]
                    fxsB = fxsB_tiles[t % 6]
                    fxp4 = fxp.rearrange("p (c h d) -> p c h d", c=4, h=2)
                    nc.scalar.copy(fxsA[:, :, :, 0:D], fxp4[:, 0:2])
                    nc.vector.tensor_copy(fxsB[:, :, :, 0:D], fxp4[:, 2:4])
                    return fxsA, fxsB

                def emit_st(td, fxsA_d, fxsB_d):
                    swn2_d = swn_tiles[(td // 2) % 4][:, td % 2]
                    for c in range(4):
                        src = fxsA_d if c < 2 else fxsB_d
                        nc.tensor.matmul(
                            st_ps[c // 2][:, c % 2, :],
                            swn2_d[:, ts(c, 128)],
                            src[:, c % 2].rearrange("p a b -> p (a b)"),
                            start=(td == 0 and c % 2 == 0),
                            stop=(td == T - 1 and c % 2 == 1))

                fxs_hist = {}
                xt4_hist = {}

                def load_quad(q0):
                    # 8-tile input load on the gpsimd software queue (keeps
                    # both hw queues clear: sync has the transposes, scalar
                    # is compute-choked)
                    xt4 = work.tile([128, 2, 8 * TOK], BF16, tag="xt4")
                    nc.gpsimd.dma_start(xt4, xT_d[:, :, q0 * TOK:(q0 + 8) * TOK])
                    xt4_hist[q0] = xt4

                load_quad(0)
                load_quad(8)
                for t0 in range(0, T + ST_DELAY, 2):
                    if t0 % 8 == 0 and t0 + 16 < T:
                        load_quad(t0 + 16)
                    if t0 < T:
                        xt4 = xt4_hist[t0 - t0 % 8]
                        if t0 % 8 == 6:
                            del xt4_hist[t0 - 6]
                        # PE burst: mains for both tiles of the pair
                        for t in (t0, t0 + 1):
                            xt = xt4[:, :, (t % 8) * TOK:(t % 8 + 1) * TOK]
                            lg = psmm.tile([128, H * G], F32, tag="lg")
                            fxp = psmm.tile([128, INNER], F32, tag="fx")
                            for k in range(2):
                                nc.tensor.matmul(lg, xt[:, k, :], AT_sb[:, k, :],
                                                 start=(k == 0), stop=(k == 1))
                                nc.tensor.matmul(fxp, xt[:, k, :], WfxT_sb[:, k, :],
                                                 start=(k == 0), stop=(k == 1))
                            hist[t] = (lg, fxp)
                        for t in (t0, t0 + 1):
                            emit_chain(t)
                            fxs_hist[t] = emit_fxs(t)

                    # delayed slice-token accumulation (PE burst part 2)
                    for td in (t0 - ST_DELAY, t0 - ST_DELAY + 1):
                        if 0 <= td < T and td in fxs_hist:
                            emit_st(td, *fxs_hist.pop(td))

                # drain remaining delayed st matmuls
                for td in sorted(fxs_hist):
                    emit_st(td, *fxs_hist.pop(td))

                # ========== STAGE (slice attention, tiny) ==========
                # copy accumulators to SBUF (partition-aligned)
                stA = [stg_pool.tile([128, 2, 2 * (D + 1)], F32, name=f"stA{j}")
                       for j in range(2)]
                nc.vector.tensor_copy(stA[0], st_ps[0])
                nc.vector.tensor_copy(stA[1], st_ps[1])

                cc_in = dram.tile([64, H * (D + 1)], F32)
                cc_out = dram.tile([64, H * (D + 1)], F32)
                # head index h = 4j + 2k + b
                cc_in_v = cc_in.rearrange("p (j k b e) -> p j k b e", j=2, k=2, b=2)
                # head 2c   = stA[c//2][0:64,  c%2, 0:65]   (c = 2j + k)
                # head 2c+1 = stA[c//2][64:128, c%2, 65:130]
                for j in range(2):
                    nc.sync.dma_start(cc_in_v[:, j, :, 0, :],
                                      stA[j][0:64, :, 0:D + 1])
                    nc.sync.dma_start(cc_in_v[:, j, :, 1, :],
                                      stA[j][64:128, :, D + 1:2 * (D + 1)])
                nc.gpsimd.collective_compute(
                    "AllReduce", ALU.add,
                    replica_groups=[[0, 1], [2, 3], [4, 5], [6, 7]],
                    ins=[cc_in.opt()], outs=[cc_out.opt()],
                )
                stg = stg_pool.tile([64, H, D + 1], F32)
                nc.sync.dma_start(stg.rearrange("p h e -> p (h e)"), cc_out)

                snorm_e = stg_pool.tile([64, H], F32)
                nc.vector.tensor_scalar_add(snorm_e, stg[:, :, D], EPS_SLICE)
                rs = stg_pool.tile([64, H], F32)
                nc.vector.reciprocal(rs, snorm_e)
                st_sb = stg_pool.tile([64, H, D], F32)
                nc.vector.tensor_tensor(st_sb, stg[:, :, 0:D],
                                        rs[:, :, None].to_broadcast([64, H, D]),
                                        ALU.mult)
                kv = stg_pool.tile([64, D], F32)
                nc.vector.reduce_sum(kv, st_sb.rearrange("p h d -> p d h"),
                                     axis=mybir.AxisListType.X)

                # transposes of st and kv (f32, 64x64)
                stT = stg_pool.tile([64, H, D], F32)
                for h in range(H):
                    tp = psmm.tile([64, 64], F32, tag="fx")
                    nc.tensor.transpose(tp, st_sb[:, h, :], id32)
                    nc.vector.tensor_copy(stT[:, h, :], tp)
                kvT_p = psmm.tile([64, 64], F32, tag="fx")
                nc.tensor.transpose(kvT_p, kv, id32)
                kvT = stg_pool.tile([64, D], F32)
                nc.vector.tensor_copy(kvT, kvT_p)

                # q = st @ WqT (per head), k/v from kv
                q_ps = psmm.tile([64, H, D], F32, tag="lg")
                for h in range(H):
                    nc.tensor.matmul(q_ps[:, h, :], stT[:, h, :], WqT_sb,
                                     start=(h == 0), stop=(h == H - 1))
                k_ps = psmm.tile([64, D], F32, tag="fx")
                nc.tensor.matmul(k_ps, kvT, WkT_sb, start=True, stop=True)
                v_ps = psmm.tile([64, D], F32, tag="fx")
                nc.tensor.matmul(v_ps, kvT, WvT_sb, start=True, stop=True)
                v_sb = stg_pool.tile([64, D], F32)
                nc.vector.tensor_copy(v_sb, v_ps)

                def rnorm(src_ps, nh, tag):
                    # 1/sqrt(sum(src^2 over last dim)) with one Newton step
                    sq = stg_pool.tile([64, nh, D], F32, name=f"sq_{tag}")
                    nc.scalar.activation(sq, src_ps, AF.Square)
                    n2 = stg_pool.tile([64, nh], F32, name=f"n2_{tag}")
                    nc.vector.reduce_sum(n2, sq, axis=mybir.AxisListType.X)
                    r0 = stg_pool.tile([64, nh], F32, name=f"r0_{tag}")
                    nc.vector.reciprocal(r0, n2)
                    y0 = stg_pool.tile([64, nh], F32, name=f"y0_{tag}")
                    nc.scalar.activation(y0, r0, AF.Sqrt)
                    t1 = stg_pool.tile([64, nh], F32, name=f"t1_{tag}")
                    nc.vector.tensor_mul(t1, y0, y0)
                    nc.vector.tensor_mul(t1, t1, n2)
                    nc.vector.tensor_scalar(t1, t1, -0.5, 1.5, ALU.mult, ALU.add)
                    nc.vector.tensor_mul(t1, t1, y0)
                    return t1

                rq = rnorm(q_ps, H, "q")
                rk = rnorm(k_ps[:, None, :], 1, "k")

                qn = stg_pool.tile([64, H, D], F32)
                nc.vector.tensor_tensor(qn, q_ps,
                                        rq[:, :, None].to_broadcast([64, H, D]),
                                        ALU.mult)
                kn = stg_pool.tile([64, D], F32)
                nc.vector.tensor_tensor(kn, k_ps,
                                        rk[:, 0:1].to_broadcast([64, D]), ALU.mult)

                qnT = stg_pool.tile([64, H, D], F32)
                for h in range(H):
                    tp = psmm.tile([64, 64], F32, tag="fx")
                    nc.tensor.transpose(tp, qn[:, h, :], id32)
                    nc.vector.tensor_copy(qnT[:, h, :], tp)
                knT_p = psmm.tile([64, 64], F32, tag="fx")
                nc.tensor.transpose(knT_p, kn, id32)
                knT = stg_pool.tile([64, D], F32)
                nc.vector.tensor_copy(knT, knT_p)

                # attention logits both orientations, exp, denominators
                L_ps = psmm.tile([64, H, G], F32, tag="lg")
                for h in range(H):
                    nc.tensor.matmul(L_ps[:, h, :], qnT[:, h, :], knT,
                                     start=(h == 0), stop=(h == H - 1))
                e_sb = stg_pool.tile([64, H, G], F32)
                nc.scalar.activation(e_sb, L_ps, AF.Exp, scale=attn_scale)
                aden = stg_pool.tile([64, H], F32)
                nc.vector.reduce_sum(aden, e_sb, axis=mybir.AxisListType.X)
                ra = stg_pool.tile([64, H], F32)
                nc.vector.reciprocal(ra, aden)

                LT_ps = psmm.tile([64, H, G], F32, tag="fx")
                for h in range(H):
                    nc.tensor.matmul(LT_ps[:, h, :], knT, qnT[:, h, :],
                                     start=(h == 0), stop=(h == H - 1))
                eT_sb = stg_pool.tile([64, H, G], F32)
                nc.scalar.activation(eT_sb, LT_ps, AF.Exp, scale=attn_scale)

                av_ps = psmm.tile([64, H, D], F32, tag="lg")
                for h in range(H):
                    nc.tensor.matmul(av_ps[:, h, :], eT_sb[:, h, :], v_sb,
                                     start=(h == 0), stop=(h == H - 1))

                os_sb = stg_pool.tile([64, H, D], F32)
                nc.vector.tensor_tensor(os_sb, av_ps,
                                        ra[:, :, None].to_broadcast([64, H, D]),
                                        ALU.mult)
                rst = stg_pool.tile([64, H, D], F32)
                nc.vector.tensor_scalar_mul(rst, st_sb, res_scale)
                nc.vector.tensor_add(os_sb, os_sb, rst)

                osT = stg_pool.tile([64, H, D], BF16)
                for h in range(H):
                    tp = psmm.tile([64, 64], F32, tag="fx")
                    nc.tensor.transpose(tp, os_sb[:, h, :], id32)
                    nc.vector.tensor_copy(osT[:, h, :], tp)

                for j in range(4):
                    C_ps = psmm.tile([128, DIM], F32, tag="lg")
                    for par in range(2):
                        h = 2 * j + par
                        nc.tensor.matmul(C_ps[64 * par:64 * par + 64, :],
                                         osT[:, h, :], WoT_sb[:, h, :],
                                         start=True, stop=True)
                    nc.vector.tensor_copy(C_sb[:, j, :], C_ps)

            # ================= PASS 2 =================
            # transposed-output formulation: for each 4-tile group, compute
            # outT[f, tok] = sum_c C[c-chunk, f].T @ swT[c-chunk, 4 tiles of
            # tok] with a 512-wide moving operand — half the matmuls of the
            # [tok, f] orientation. The host transposes back.
            with tc.tile_pool(name="ps2", bufs=3, space="PSUM") as ps2:
                ob = None
                for g in range(T // 4):
                    t0 = 4 * g
                    opa = ps2.tile([128, 4 * TOK], F32, tag="p2a")
                    opb = ps2.tile([128, 4 * TOK], F32, tag="p2b")
                    for c in range(4):
                        rhs = swT_store[:, t0:t0 + 4, c, :]
                        nc.tensor.matmul(opa, C_sb[:, c, 0:128], rhs,
                                         start=(c == 0), stop=(c == 3))
                        nc.tensor.matmul(opb, C_sb[:, c, 128:256], rhs,
                                         start=(c == 0), stop=(c == 3))
                    if g % 2 == 0:
                        ob = obuf.tile([128, 2, 2, 4 * TOK], BF16, tag="ob")
                        nc.scalar.copy(ob[:, 0, 0], opa)
                        nc.vector.tensor_copy(ob[:, 1, 0], opb)
                    else:
                        nc.vector.tensor_copy(ob[:, 0, 1], opa)
                        nc.scalar.copy(ob[:, 1, 1], opb)
                        eng = nc.sync if (g // 2) % 2 == 0 else nc.scalar
                        eng.dma_start(
                            outT_v[:, :, (t0 - 4) * TOK:(t0 + 4) * TOK],
                            ob.rearrange("p fc g j -> p fc (g j)"))

    nc.finalize()
    return nc


def kernel(x, Wfx, bfx, Wx, bx, Wslice, bslice, temp, Wq, Wk, Wv,
           res_scale, attn_scale, Wout, bout):
    x = np.asarray(x, dtype=np.float32)
    Wfx = np.asarray(Wfx, np.float32); bfx = np.asarray(bfx, np.float32)
    Wx = np.asarray(Wx, np.float32); bx = np.asarray(bx, np.float32)
    Wslice = np.asarray(Wslice, np.float32); bslice = np.asarray(bslice, np.float32)
    temp = np.asarray(temp, np.float32).reshape(H)
    Wq = np.asarray(Wq, np.float32); Wk = np.asarray(Wk, np.float32)
    Wv = np.asarray(Wv, np.float32)
    res_scale_f = float(np.asarray(res_scale, np.float32))
    attn = np.asarray(attn_scale, np.float32).reshape(H)
    Wout = np.asarray(Wout, np.float32); bout = np.asarray(bout, np.float32)

    assert np.all(np.abs(bfx) == 0) and np.all(np.abs(bx) == 0) \
        and np.all(np.abs(bslice) == 0), "nonzero projection biases unsupported"
    assert np.ptp(attn) == 0, "non-uniform attn_scale unsupported"
    attn_f = float(attn[0])

    # folded logits weight: logits[:, h*G+g] = x @ ((Wslice @ Wx_h)/temp_h).T
    A = np.concatenate(
        [(Wslice @ Wx[h * D:(h + 1) * D, :]) / temp[h] for h in range(H)], axis=0)
    BFNP = ml_dtypes.bfloat16

    def chunk_major(w):  # [256, cols] -> [128, 2, cols]
        return np.ascontiguousarray(w.reshape(2, 128, -1).transpose(1, 0, 2))

    AT = chunk_major(A.T.astype(BFNP))                    # [128, 2, 512]
    WfxT = chunk_major(Wfx.T.astype(BFNP))                # [128, 2, 512]
    WoT = np.ascontiguousarray(Wout.T).astype(ml_dtypes.bfloat16)  # [512, 256]
    WqT = np.ascontiguousarray(Wq.T)
    WkT = np.ascontiguousarray(Wk.T) / H
    WvT = np.ascontiguousarray(Wv.T) / H
    id32 = np.eye(64, dtype=np.float32)

    key = (attn_f, res_scale_f)
    if key not in _CACHE:
        _CACHE[key] = _build(attn_f, res_scale_f)
    nc = _CACHE[key]

    in_maps = []
    for c in range(NCORES):
        b, half = c // 2, c % 2
        xs = x[b, half * NLOC:(half + 1) * NLOC, :]       # [16384, 256]
        xT = chunk_major(np.ascontiguousarray(xs.T).astype(BFNP))  # [128,2,NLOC]
        in_maps.append(dict(xT=xT, AT=AT, WfxT=WfxT, id32=id32,
                            WqT=WqT, WkT=WkT, WvT=WvT, WoT=WoT))

    global _LAST_IN_MAPS
    _LAST_IN_MAPS = in_maps
    res = bass_utils.run_bass_kernel_spmd(nc, in_maps, core_ids=list(range(NCORES)))

    out = np.empty((B, N, DIM), np.float32)
    for c in range(NCORES):
        b, half = c // 2, c % 2
        oT = res.results[c]["outT"].reshape(DIM, NLOC)
        out[b, half * NLOC:(half + 1) * NLOC, :] = \
            np.ascontiguousarray(oT.T).astype(np.float32)
    if np.any(bout):
        out += bout
    return out


# revision 34
# speedup vs baseline: 1.1094x; 1.1094x over previous
"""Trainium2 Bass kernel for Physics-Attention over an irregular mesh.

Contract: kernel(**inputs) takes the FULL inputs from setup_inputs() and
returns the FULL [4, 32768, 256] f32 output, distributing across 8 cores
internally (one (batch, half-of-N) shard per core, pairwise AllReduce on the
slice-token pooling reductions).

Structure (single core):
  pass 1 (per 128-token tile): logits/fx matmuls (bf16, chunked K=256),
    exp -> per-head denom -> reciprocal -> normalized routing weights swn;
    slice-token accumulation via pair-merged fp32-accumulating matmuls,
    software-pipelined ST_DELAY tiles behind the mains so the PE stays in
    long bursts (high p-state); swn is transposed for pass 2 by the DMA
    XBAR (sync queue), input loads ride the gpsimd software queue.
  stage: pairwise AllReduce of slice-token partials, tiny slice attention.
  pass 2: out = swT @ C in 8-tile supertiles, psum-bank-rotated matmuls,
    bf16 output written via batched DMA (host upcasts to f32).
"""

import sys

sys.path.insert(0, "/opt/trn_rl_repo")

import numpy as np
import ml_dtypes

import concourse.bass as bass
import concourse.mybir as mybir
import concourse.tile as tile
from concourse import bacc, bass_utils
from concourse.bass import ts

F32 = mybir.dt.float32
BF16 = mybir.dt.bfloat16
AF = mybir.ActivationFunctionType
ALU = mybir.AluOpType

B, N, DIM = 4, 32768, 256
H, D, G = 8, 64, 64
INNER = H * D  # 512
NCORES = 8
NLOC = N // 2          # 16384 tokens per core
TOK = 128              # tokens per tile
T = NLOC // TOK        # 128 tiles
EPS_SLICE = 1e-5
ST_DELAY = 4           # software-pipeline delay of the st matmuls (tiles)
SUP = 8                # pass-2 supertile (tiles per output DMA)

_CACHE = {}


def _build(attn_scale: float, res_scale: float, debug: bool = False):
    """Build the single-core SPMD program (identical on all 8 cores)."""
    nc = bacc.Bacc("TRN2", target_bir_lowering=False, debug=False,
                   enable_asserts=False, num_devices=NCORES)

    xT_d = nc.dram_tensor("xT", [128, 2, NLOC], BF16, kind="ExternalInput").ap()
    AT_d = nc.dram_tensor("AT", [128, 2, INNER], BF16, kind="ExternalInput").ap()
    WfxT_d = nc.dram_tensor("WfxT", [128, 2, INNER], BF16, kind="ExternalInput").ap()
    id32_d = nc.dram_tensor("id32", [64, 64], F32, kind="ExternalInput").ap()
    WqT_d = nc.dram_tensor("WqT", [D, D], F32, kind="ExternalInput").ap()
    WkT_d = nc.dram_tensor("WkT", [D, D], F32, kind="ExternalInput").ap()
    WvT_d = nc.dram_tensor("WvT", [D, D], F32, kind="ExternalInput").ap()
    WoT_d = nc.dram_tensor("WoT", [INNER, DIM], BF16, kind="ExternalInput").ap()
    outT_d = nc.dram_tensor("outT", [2, 128, NLOC], BF16, kind="ExternalOutput").ap()

    WoT_v = WoT_d.rearrange("(h d) f -> d h f", d=64)   # [64, 8, 256]
    # transposed output view: [p, fc, n] for one DMA per 4-tile group
    outT_v = outT_d.rearrange("fc p n -> p fc n")

    with tile.TileContext(nc) as tc:
        with (
            tc.tile_pool(name="consts", bufs=1) as consts,
            tc.tile_pool(name="store", bufs=1) as store,
            tc.tile_pool(name="work", bufs=3) as work,
            tc.tile_pool(name="uswp", bufs=8) as uswp,
            tc.tile_pool(name="small", bufs=6) as small,
            tc.tile_pool(name="stage", bufs=1) as stg_pool,
            tc.tile_pool(name="obuf", bufs=2) as obuf,
            tc.tile_pool(name="dram", bufs=1, space="DRAM") as dram,
        ):
            # ---- resident constants ----
            AT_sb = consts.tile([128, 2, INNER], BF16)
            nc.sync.dma_start(AT_sb, AT_d)
            WfxT_sb = consts.tile([128, 2, INNER], BF16)
            nc.sync.dma_start(WfxT_sb, WfxT_d)
            id32 = consts.tile([64, 64], F32)
            nc.sync.dma_start(id32, id32_d)
            WqT_sb = consts.tile([64, 64], F32)
            nc.sync.dma_start(WqT_sb, WqT_d)
            WkT_sb = consts.tile([64, 64], F32)
            nc.sync.dma_start(WkT_sb, WkT_d)
            WvT_sb = consts.tile([64, 64], F32)
            nc.sync.dma_start(WvT_sb, WvT_d)
            WoT_sb = consts.tile([64, H, DIM], BF16)
            nc.sync.dma_start(WoT_sb, WoT_v)

            # transposed routing weights, written by DMA transpose:
            # swT_store[p, t, c, j] = swn_t[j, c*128 + p]
            swT_store = store.tile([128, T, 4, TOK], BF16)

            # persistent swn pair tiles (4-deep manual rotation): pair P holds
            # tiles 2P, 2P+1; DMA-transposed together after the odd tile
            swn_tiles = [consts.tile([128, 2, H * G], BF16, name=f"swn{i}")
                         for i in range(4)]

            # persistent fxs tiles (6-deep manual rotation), ones columns
            # preset once: fxs*[p, c, half, 0:64] = fx data, [.., 64] = 1.
            # Split into a scalar-written set (pairs 0-1) and a vector-written
            # set (pairs 2-3) so the two engines share no tile (avoids false
            # WAW lockstep).
            fxsA_tiles = [consts.tile([128, 2, 2, D + 1], BF16, name=f"fxsA{i}")
                          for i in range(6)]
            fxsB_tiles = [consts.tile([128, 2, 2, D + 1], BF16, name=f"fxsB{i}")
                          for i in range(6)]
            for i in range(6):
                nc.gpsimd.memset(fxsA_tiles[i][:, :, :, D], 1.0)
                nc.gpsimd.memset(fxsB_tiles[i][:, :, :, D], 1.0)

            C_sb = stg_pool.tile([128, 4, DIM], BF16)

            with (
                tc.tile_pool(name="psmm", bufs=3, space="PSUM") as psmm,
                tc.tile_pool(name="psacc", bufs=1, space="PSUM") as psacc,
            ):
                # slice-token accumulators: pair c = heads (2c, 2c+1) lives in
                # st_ps[c//2][:, c%2, :]; valid regions: head 2c ->
                # [0:64, 0:65], head 2c+1 -> [64:128, 65:130].
                st_ps = [psacc.tile([128, 2, 2 * (D + 1)], F32, name=f"st_ps{j}")
                         for j in range(2)]

                # ================= PASS 1 =================
                # processed in tile PAIRS: PE does mains for both tiles
                # back-to-back, then the delayed st matmuls for an earlier
                # pair — long PE bursts keep the tensor engine at high pstate.
                # The elementwise chain (exp/reduce/recip/mult) is fused at
                # pair granularity to amortize fixed costs; fxs casts are
                # emitted after the chain so they can't block the next exp.
                hist = {}  # tile t -> (lg, fxp) psum handles

                def emit_chain(t):
                    lg, fxp = hist[t]
                    usw = uswp.tile([128, H, G], BF16, tag="usw")
                    nc.scalar.activation(usw, lg.rearrange("p (h g) -> p h g", h=H),
                                         AF.Exp)
                    den = small.tile([128, H], BF16, tag="den")
                    rden = small.tile([128, H], BF16, tag="rden")
                    with nc.allow_low_precision(reason="softmax denom in bf16"):
                        nc.vector.reduce_sum(den, usw, axis=mybir.AxisListType.X)
                        nc.vector.reciprocal(rden, den)
                    swn_pair = swn_tiles[(t // 2) % 4]
                    swn = swn_pair[:, t % 2].rearrange("p (h g) -> p h g", h=H)
                    nc.gpsimd.tensor_tensor(
                        swn, usw, rden[:, :, None].to_broadcast([128, H, G]),
                        ALU.mult)
                    if t % 2 == 1:
                        # transposed copy for pass 2, on the DMA engines
                        nc.sync.dma_start(
                            swT_store[:, t - 1:t + 1].rearrange(
                                "p a b j -> p (a b) j"),
                            swn_pair.rearrange("p a f -> p (a f)"),
                            transpose=True)

                def emit_fxs(t):
                    _, fxp = hist.pop(t)
                    fxsA = fxsA_tiles[t % 6]
                    fxsB = fxsB_tiles[t % 6]
                    fxp4 = fxp.rearrange("p (c h d) -> p c h d", c=4, h=2)
                    nc.scalar.copy(fxsA[:, :, :, 0:D], fxp4[:, 0:2])
                    nc.vector.tensor_copy(fxsB[:, :, :, 0:D], fxp4[:, 2:4])
                    return fxsA, fxsB

                def emit_st(td, fxsA_d, fxsB_d):
                    swn2_d = swn_tiles[(td // 2) % 4][:, td % 2]
                    for c in range(4):
                        src = fxsA_d if c < 2 else fxsB_d
                        nc.tensor.matmul(
                            st_ps[c // 2][:, c % 2, :],
                            swn2_d[:, ts(c, 128)],
                            src[:, c % 2].rearrange("p a b -> p (a b)"),
                            start=(td == 0 and c % 2 == 0),
                            stop=(td == T - 1 and c % 2 == 1))

                fxs_hist = {}
                xt4_hist = {}

                def load_quad(q0):
                    # 8-tile input load on the gpsimd software queue (keeps
                    # both hw queues clear: sync has the transposes, scalar
                    # is compute-choked)
                    xt4 = work.tile([128, 2, 8 * TOK], BF16, tag="xt4")
                    nc.gpsimd.dma_start(xt4, xT_d[:, :, q0 * TOK:(q0 + 8) * TOK])
                    xt4_hist[q0] = xt4

                load_quad(0)
                load_quad(8)
                for t0 in range(0, T + ST_DELAY, 2):
                    if t0 % 8 == 0 and t0 + 16 < T:
                        load_quad(t0 + 16)
                    if t0 < T:
                        xt4 = xt4_hist[t0 - t0 % 8]
                        if t0 % 8 == 6:
                            del xt4_hist[t0 - 6]
                        # PE burst: mains for both tiles of the pair
                        for t in (t0, t0 + 1):
                            xt = xt4[:, :, (t % 8) * TOK:(t % 8 + 1) * TOK]
                            lg = psmm.tile([128, H * G], F32, tag="lg")
                            fxp = psmm.tile([128, INNER], F32, tag="fx")
                            for k in range(2):
                                nc.tensor.matmul(lg, xt[:, k, :], AT_sb[:, k, :],
                                                 start=(k == 0), stop=(k == 1))
                                nc.tensor.matmul(fxp, xt[:, k, :], WfxT_sb[:, k, :],
                                                 start=(k == 0), stop=(k == 1))
                            hist[t] = (lg, fxp)
                        for t in (t0, t0 + 1):
                            emit_chain(t)
                            fxs_hist[t] = emit_fxs(t)

                    # delayed slice-token accumulation (PE burst part 2)
                    for td in (t0 - ST_DELAY, t0 - ST_DELAY + 1):
                        if 0 <= td < T and td in fxs_hist:
                            emit_st(td, *fxs_hist.pop(td))

                # drain remaining delayed st matmuls
                for td in sorted(fxs_hist):
                    emit_st(td, *fxs_hist.pop(td))

                # ========== STAGE (slice attention, tiny) ==========
                # copy accumulators to SBUF (partition-aligned)
                stA = [stg_pool.tile([128, 2, 2 * (D + 1)], F32, name=f"stA{j}")
                       for j in range(2)]
                nc.vector.tensor_copy(stA[0], st_ps[0])
                nc.vector.tensor_copy(stA[1], st_ps[1])

                cc_in = dram.tile([64, H * (D + 1)], F32)
                cc_out = dram.tile([64, H * (D + 1)], F32)
                # head index h = 4j + 2k + b
                cc_in_v = cc_in.rearrange("p (j k b e) -> p j k b e", j=2, k=2, b=2)
                # head 2c   = stA[c//2][0:64,  c%2, 0:65]   (c = 2j + k)
                # head 2c+1 = stA[c//2][64:128, c%2, 65:130]
                for j in range(2):
                    nc.sync.dma_start(cc_in_v[:, j, :, 0, :],
                                      stA[j][0:64, :, 0:D + 1])
                    nc.sync.dma_start(cc_in_v[:, j, :, 1, :],
                                      stA[j][64:128, :, D + 1:2 * (D + 1)])
                nc.gpsimd.collective_compute(
                    "AllReduce", ALU.add,
                    replica_groups=[[0, 1], [2, 3], [4, 5], [6, 7]],
                    ins=[cc_in.opt()], outs=[cc_out.opt()],
                )
                stg = stg_pool.tile([64, H, D + 1], F32)
                nc.sync.dma_start(stg.rearrange("p h e -> p (h e)"), cc_out)

                snorm_e = stg_pool.tile([64, H], F32)
                nc.vector.tensor_scalar_add(snorm_e, stg[:, :, D], EPS_SLICE)
                rs = stg_pool.tile([64, H], F32)
                nc.vector.reciprocal(rs, snorm_e)
                st_sb = stg_pool.tile([64, H, D], F32)
                nc.vector.tensor_tensor(st_sb, stg[:, :, 0:D],
                                        rs[:, :, None].to_broadcast([64, H, D]),
                                        ALU.mult)
                kv = stg_pool.tile([64, D], F32)
                nc.vector.reduce_sum(kv, st_sb.rearrange("p h d -> p d h"),
                                     axis=mybir.AxisListType.X)

                # transposes of st and kv (f32, 64x64)
                stT = stg_pool.tile([64, H, D], F32)
                for h in range(H):
                    tp = psmm.tile([64, 64], F32, tag="fx")
                    nc.tensor.transpose(tp, st_sb[:, h, :], id32)
                    nc.vector.tensor_copy(stT[:, h, :], tp)
                kvT_p = psmm.tile([64, 64], F32, tag="fx")
                nc.tensor.transpose(kvT_p, kv, id32)
                kvT = stg_pool.tile([64, D], F32)
                nc.vector.tensor_copy(kvT, kvT_p)

                # q = st @ WqT (per head), k/v from kv
                q_ps = psmm.tile([64, H, D], F32, tag="lg")
                for h in range(H):
                    nc.tensor.matmul(q_ps[:, h, :], stT[:, h, :], WqT_sb,
                                     start=(h == 0), stop=(h == H - 1))
                k_ps = psmm.tile([64, D], F32, tag="fx")
                nc.tensor.matmul(k_ps, kvT, WkT_sb, start=True, stop=True)
                v_ps = psmm.tile([64, D], F32, tag="fx")
                nc.tensor.matmul(v_ps, kvT, WvT_sb, start=True, stop=True)
                v_sb = stg_pool.tile([64, D], F32)
                nc.vector.tensor_copy(v_sb, v_ps)

                def rnorm(src_ps, nh, tag):
                    # 1/sqrt(sum(src^2 over last dim)) with one Newton step
                    sq = stg_pool.tile([64, nh, D], F32, name=f"sq_{tag}")
                    nc.scalar.activation(sq, src_ps, AF.Square)
                    n2 = stg_pool.tile([64, nh], F32, name=f"n2_{tag}")
                    nc.vector.reduce_sum(n2, sq, axis=mybir.AxisListType.X)
                    r0 = stg_pool.tile([64, nh], F32, name=f"r0_{tag}")
                    nc.vector.reciprocal(r0, n2)
                    y0 = stg_pool.tile([64, nh], F32, name=f"y0_{tag}")
                    nc.scalar.activation(y0, r0, AF.Sqrt)
                    t1 = stg_pool.tile([64, nh], F32, name=f"t1_{tag}")
                    nc.vector.tensor_mul(t1, y0, y0)
                    nc.vector.tensor_mul(t1, t1, n2)
                    nc.vector.tensor_scalar(t1, t1, -0.5, 1.5, ALU.mult, ALU.add)
                    nc.vector.tensor_mul(t1, t1, y0)
                    return t1

                rq = rnorm(q_ps, H, "q")
                rk = rnorm(k_ps[:, None, :], 1, "k")

                qn = stg_pool.tile([64, H, D], F32)
                nc.vector.tensor_tensor(qn, q_ps,
                                        rq[:, :, None].to_broadcast([64, H, D]),
                                        ALU.mult)
                kn = stg_pool.tile([64, D], F32)
                nc.vector.tensor_tensor(kn, k_ps,
                                        rk[:, 0:1].to_broadcast([64, D]), ALU.mult)

                qnT = stg_pool.tile([64, H, D], F32)
                for h in range(H):
                    tp = psmm.tile([64, 64], F32, tag="fx")
                    nc.tensor.transpose(tp, qn[:, h, :], id32)
                    nc.vector.tensor_copy(qnT[:, h, :], tp)
                knT_p = psmm.tile([64, 64], F32, tag="fx")
                nc.tensor.transpose(knT_p, kn, id32)
                knT = stg_pool.tile([64, D], F32)
                nc.vector.tensor_copy(knT, knT_p)

                # attention logits both orientations, exp, denominators
                L_ps = psmm.tile([64, H, G], F32, tag="lg")
                for h in range(H):
                    nc.tensor.matmul(L_ps[:, h, :], qnT[:, h, :], knT,
                                     start=(h == 0), stop=(h == H - 1))
                e_sb = stg_pool.tile([64, H, G], F32)
                nc.scalar.activation(e_sb, L_ps, AF.Exp, scale=attn_scale)
                aden = stg_pool.tile([64, H], F32)
                nc.vector.reduce_sum(aden, e_sb, axis=mybir.AxisListType.X)
                ra = stg_pool.tile([64, H], F32)
                nc.vector.reciprocal(ra, aden)

                LT_ps = psmm.tile([64, H, G], F32, tag="fx")
                for h in range(H):
                    nc.tensor.matmul(LT_ps[:, h, :], knT, qnT[:, h, :],
                                     start=(h == 0), stop=(h == H - 1))
                eT_sb = stg_pool.tile([64, H, G], F32)
                nc.scalar.activation(eT_sb, LT_ps, AF.Exp, scale=attn_scale)

                av_ps = psmm.tile([64, H, D], F32, tag="lg")
                for h in range(H):
                    nc.tensor.matmul(av_ps[:, h, :], eT_sb[:, h, :], v_sb,
                                     start=(h == 0), stop=(h == H - 1))

                os_sb = stg_pool.tile([64, H, D], F32)
                nc.vector.tensor_tensor(os_sb, av_ps,
                                        ra[:, :, None].to_broadcast([64, H, D]),
                                        ALU.mult)
                rst = stg_pool.tile([64, H, D], F32)
                nc.vector.tensor_scalar_mul(rst, st_sb, res_scale)
                nc.vector.tensor_add(os_sb, os_sb, rst)

                osT = stg_pool.tile([64, H, D], BF16)
                for h in range(H):
                    tp = psmm.tile([64, 64], F32, tag="fx")
                    nc.tensor.transpose(tp, os_sb[:, h, :], id32)
                    nc.vector.tensor_copy(osT[:, h, :], tp)

                for j in range(4):
                    C_ps = psmm.tile([128, DIM], F32, tag="lg")
                    for par in range(2):
                        h = 2 * j + par
                        nc.tensor.matmul(C_ps[64 * par:64 * par + 64, :],
                                         osT[:, h, :], WoT_sb[:, h, :],
                                         start=True, stop=True)
                    nc.vector.tensor_copy(C_sb[:, j, :], C_ps)

            # ================= PASS 2 =================
            # transposed-output formulation: for each 4-tile group, compute
            # outT[f, tok] = sum_c C[c-chunk, f].T @ swT[c-chunk, 4 tiles of
            # tok] with a 512-wide moving operand — half the matmuls of the
            # [tok, f] orientation. The host transposes back.
            with tc.tile_pool(name="ps2", bufs=3, space="PSUM") as ps2:
                ob = None
                for g in range(T // 4):
                    t0 = 4 * g
                    opa = ps2.tile([128, 4 * TOK], F32, tag="p2a")
                    opb = ps2.tile([128, 4 * TOK], F32, tag="p2b")
                    for c in range(4):
                        rhs = swT_store[:, t0:t0 + 4, c, :]
                        nc.tensor.matmul(opa, C_sb[:, c, 0:128], rhs,
                                         start=(c == 0), stop=(c == 3))
                        nc.tensor.matmul(opb, C_sb[:, c, 128:256], rhs,
                                         start=(c == 0), stop=(c == 3))
                    if g % 2 == 0:
                        ob = obuf.tile([128, 2, 2, 4 * TOK], BF16, tag="ob")
                        nc.scalar.copy(ob[:, 0, 0], opa)
                        nc.vector.tensor_copy(ob[:, 1, 0], opb)
                    else:
                        nc.vector.tensor_copy(ob[:, 0, 1], opa)
                        nc.scalar.copy(ob[:, 1, 1], opb)
                        nc.sync.dma_start(
                            outT_v[:, :, (t0 - 4) * TOK:(t0 + 4) * TOK],
                            ob.rearrange("p fc g j -> p fc (g j)"))

    nc.finalize()
    return nc


def kernel(x, Wfx, bfx, Wx, bx, Wslice, bslice, temp, Wq, Wk, Wv,
           res_scale, attn_scale, Wout, bout):
    x = np.asarray(x, dtype=np.float32)
    Wfx = np.asarray(Wfx, np.float32); bfx = np.asarray(bfx, np.float32)
    Wx = np.asarray(Wx, np.float32); bx = np.asarray(bx, np.float32)
    Wslice = np.asarray(Wslice, np.float32); bslice = np.asarray(bslice, np.float32)
    temp = np.asarray(temp, np.float32).reshape(H)
    Wq = np.asarray(Wq, np.float32); Wk = np.asarray(Wk, np.float32)
    Wv = np.asarray(Wv, np.float32)
    res_scale_f = float(np.asarray(res_scale, np.float32))
    attn = np.asarray(attn_scale, np.float32).reshape(H)
    Wout = np.asarray(Wout, np.float32); bout = np.asarray(bout, np.float32)

    assert np.all(np.abs(bfx) == 0) and np.all(np.abs(bx) == 0) \
        and np.all(np.abs(bslice) == 0), "nonzero projection biases unsupported"
    assert np.ptp(attn) == 0, "non-uniform attn_scale unsupported"
    attn_f = float(attn[0])

    # folded logits weight: logits[:, h*G+g] = x @ ((Wslice @ Wx_h)/temp_h).T
    A = np.concatenate(
        [(Wslice @ Wx[h * D:(h + 1) * D, :]) / temp[h] for h in range(H)], axis=0)
    BFNP = ml_dtypes.bfloat16

    def chunk_major(w):  # [256, cols] -> [128, 2, cols]
        return np.ascontiguousarray(w.reshape(2, 128, -1).transpose(1, 0, 2))

    AT = chunk_major(A.T.astype(BFNP))                    # [128, 2, 512]
    WfxT = chunk_major(Wfx.T.astype(BFNP))                # [128, 2, 512]
    WoT = np.ascontiguousarray(Wout.T).astype(ml_dtypes.bfloat16)  # [512, 256]
    WqT = np.ascontiguousarray(Wq.T)
    WkT = np.ascontiguousarray(Wk.T) / H
    WvT = np.ascontiguousarray(Wv.T) / H
    id32 = np.eye(64, dtype=np.float32)

    key = (attn_f, res_scale_f)
    if key not in _CACHE:
        _CACHE[key] = _build(attn_f, res_scale_f)
    nc = _CACHE[key]

    in_maps = []
    for c in range(NCORES):
        b, half = c // 2, c % 2
        xs = x[b, half * NLOC:(half + 1) * NLOC, :]       # [16384, 256]
        xT = chunk_major(np.ascontiguousarray(xs.T).astype(BFNP))  # [128,2,NLOC]
        in_maps.append(dict(xT=xT, AT=AT, WfxT=WfxT, id32=id32,
                            WqT=WqT, WkT=WkT, WvT=WvT, WoT=WoT))

    global _LAST_IN_MAPS
    _LAST_IN_MAPS = in_maps
    res = bass_utils.run_bass_kernel_spmd(nc, in_maps, core_ids=list(range(NCORES)))

    out = np.empty((B, N, DIM), np.float32)
    for c in range(NCORES):
        b, half = c // 2, c % 2
        oT = res.results[c]["outT"].reshape(DIM, NLOC)
        out[b, half * NLOC:(half + 1) * NLOC, :] = \
            np.ascontiguousarray(oT.T).astype(np.float32)
    if np.any(bout):
        out += bout
    return out


# revision 35
# speedup vs baseline: 1.1148x; 1.0050x over previous
"""Trainium2 Bass kernel for Physics-Attention over an irregular mesh.

Contract: kernel(**inputs) takes the FULL inputs from setup_inputs() and
returns the FULL [4, 32768, 256] f32 output, distributing across 8 cores
internally (one (batch, half-of-N) shard per core, pairwise AllReduce on the
slice-token pooling reductions).

Structure (single core):
  pass 1 (per 128-token tile): logits/fx matmuls (bf16, chunked K=256),
    exp -> per-head denom -> reciprocal -> normalized routing weights swn;
    slice-token accumulation via pair-merged fp32-accumulating matmuls,
    software-pipelined ST_DELAY tiles behind the mains so the PE stays in
    long bursts (high p-state); swn is transposed for pass 2 by the DMA
    XBAR (sync queue), input loads ride the gpsimd software queue.
  stage: pairwise AllReduce of slice-token partials, tiny slice attention.
  pass 2: out = swT @ C in 8-tile supertiles, psum-bank-rotated matmuls,
    bf16 output written via batched DMA (host upcasts to f32).
"""

import sys

sys.path.insert(0, "/opt/trn_rl_repo")

import numpy as np
import ml_dtypes

import concourse.bass as bass
import concourse.mybir as mybir
import concourse.tile as tile
from concourse import bacc, bass_utils
from concourse.bass import ts

F32 = mybir.dt.float32
BF16 = mybir.dt.bfloat16
AF = mybir.ActivationFunctionType
ALU = mybir.AluOpType

B, N, DIM = 4, 32768, 256
H, D, G = 8, 64, 64
INNER = H * D  # 512
NCORES = 8
NLOC = N // 2          # 16384 tokens per core
TOK = 128              # tokens per tile
T = NLOC // TOK        # 128 tiles
EPS_SLICE = 1e-5
ST_DELAY = 4           # software-pipeline delay of the st matmuls (tiles)
SUP = 8                # pass-2 supertile (tiles per output DMA)

_CACHE = {}


def _build(attn_scale: float, res_scale: float, debug: bool = False):
    """Build the single-core SPMD program (identical on all 8 cores)."""
    nc = bacc.Bacc("TRN2", target_bir_lowering=False, debug=False,
                   enable_asserts=False, num_devices=NCORES)

    xT_d = nc.dram_tensor("xT", [128, 2, NLOC], BF16, kind="ExternalInput").ap()
    AT_d = nc.dram_tensor("AT", [128, 2, INNER], BF16, kind="ExternalInput").ap()
    WfxT_d = nc.dram_tensor("WfxT", [128, 2, INNER], BF16, kind="ExternalInput").ap()
    id32_d = nc.dram_tensor("id32", [64, 64], F32, kind="ExternalInput").ap()
    WqT_d = nc.dram_tensor("WqT", [D, D], F32, kind="ExternalInput").ap()
    WkT_d = nc.dram_tensor("WkT", [D, D], F32, kind="ExternalInput").ap()
    WvT_d = nc.dram_tensor("WvT", [D, D], F32, kind="ExternalInput").ap()
    WoT_d = nc.dram_tensor("WoT", [INNER, DIM], BF16, kind="ExternalInput").ap()
    outT_d = nc.dram_tensor("outT", [2, 128, NLOC], BF16, kind="ExternalOutput").ap()

    WoT_v = WoT_d.rearrange("(h d) f -> d h f", d=64)   # [64, 8, 256]
    # transposed output view: [p, fc, n] for one DMA per 4-tile group
    outT_v = outT_d.rearrange("fc p n -> p fc n")

    with tile.TileContext(nc) as tc:
        with (
            tc.tile_pool(name="consts", bufs=1) as consts,
            tc.tile_pool(name="store", bufs=1) as store,
            tc.tile_pool(name="work", bufs=3) as work,
            tc.tile_pool(name="uswp", bufs=8) as uswp,
            tc.tile_pool(name="small", bufs=6) as small,
            tc.tile_pool(name="stage", bufs=1) as stg_pool,
            tc.tile_pool(name="obuf", bufs=2) as obuf,
            tc.tile_pool(name="dram", bufs=1, space="DRAM") as dram,
        ):
            # ---- resident constants ----
            AT_sb = consts.tile([128, 2, INNER], BF16)
            nc.sync.dma_start(AT_sb, AT_d)
            WfxT_sb = consts.tile([128, 2, INNER], BF16)
            nc.sync.dma_start(WfxT_sb, WfxT_d)
            id32 = consts.tile([64, 64], F32)
            nc.sync.dma_start(id32, id32_d)
            WqT_sb = consts.tile([64, 64], F32)
            nc.sync.dma_start(WqT_sb, WqT_d)
            WkT_sb = consts.tile([64, 64], F32)
            nc.sync.dma_start(WkT_sb, WkT_d)
            WvT_sb = consts.tile([64, 64], F32)
            nc.sync.dma_start(WvT_sb, WvT_d)
            WoT_sb = consts.tile([64, H, DIM], BF16)
            nc.sync.dma_start(WoT_sb, WoT_v)

            # transposed routing weights, written by DMA transpose:
            # swT_store[p, t, c, j] = swn_t[j, c*128 + p]
            swT_store = store.tile([128, T, 4, TOK], BF16)

            # persistent swn pair tiles (4-deep manual rotation): pair P holds
            # tiles 2P, 2P+1; DMA-transposed together after the odd tile
            swn_tiles = [consts.tile([128, 2, H * G], BF16, name=f"swn{i}")
                         for i in range(4)]

            # persistent fxs tiles (6-deep manual rotation), ones columns
            # preset once: fxs*[p, c, half, 0:64] = fx data, [.., 64] = 1.
            # Split into a scalar-written set (pairs 0-1) and a vector-written
            # set (pairs 2-3) so the two engines share no tile (avoids false
            # WAW lockstep).
            fxsA_tiles = [consts.tile([128, 2, 2, D + 1], BF16, name=f"fxsA{i}")
                          for i in range(6)]
            fxsB_tiles = [consts.tile([128, 2, 2, D + 1], BF16, name=f"fxsB{i}")
                          for i in range(6)]
            for i in range(6):
                nc.gpsimd.memset(fxsA_tiles[i][:, :, :, D], 1.0)
                nc.gpsimd.memset(fxsB_tiles[i][:, :, :, D], 1.0)

            C_sb = stg_pool.tile([128, 4, DIM], BF16)

            with (
                tc.tile_pool(name="psmm", bufs=3, space="PSUM") as psmm,
                tc.tile_pool(name="psacc", bufs=1, space="PSUM") as psacc,
            ):
                # slice-token accumulators: pair c = heads (2c, 2c+1) lives in
                # st_ps[c//2][:, c%2, :]; valid regions: head 2c ->
                # [0:64, 0:65], head 2c+1 -> [64:128, 65:130].
                st_ps = [psacc.tile([128, 2, 2 * (D + 1)], F32, name=f"st_ps{j}")
                         for j in range(2)]

                # ================= PASS 1 =================
                # processed in tile PAIRS: PE does mains for both tiles
                # back-to-back, then the delayed st matmuls for an earlier
                # pair — long PE bursts keep the tensor engine at high pstate.
                # The elementwise chain (exp/reduce/recip/mult) is fused at
                # pair granularity to amortize fixed costs; fxs casts are
                # emitted after the chain so they can't block the next exp.
                hist = {}  # tile t -> (lg, fxp) psum handles

                def emit_chain(t):
                    lg, fxp = hist[t]
                    usw = uswp.tile([128, H, G], BF16, tag="usw")
                    nc.scalar.activation(usw, lg.rearrange("p (h g) -> p h g", h=H),
                                         AF.Exp)
                    den = small.tile([128, H], BF16, tag="den")
                    rden = small.tile([128, H], BF16, tag="rden")
                    with nc.allow_low_precision(reason="softmax denom in bf16"):
                        nc.vector.reduce_sum(den, usw, axis=mybir.AxisListType.X)
                        nc.vector.reciprocal(rden, den)
                    swn_pair = swn_tiles[(t // 2) % 4]
                    swn = swn_pair[:, t % 2].rearrange("p (h g) -> p h g", h=H)
                    nc.gpsimd.tensor_tensor(
                        swn, usw, rden[:, :, None].to_broadcast([128, H, G]),
                        ALU.mult)
                    if t % 2 == 1:
                        # transposed copy for pass 2, on the DMA engines
                        nc.sync.dma_start(
                            swT_store[:, t - 1:t + 1].rearrange(
                                "p a b j -> p (a b) j"),
                            swn_pair.rearrange("p a f -> p (a f)"),
                            transpose=True)

                def emit_fxs(t):
                    _, fxp = hist.pop(t)
                    fxsA = fxsA_tiles[t % 6]
                    fxsB = fxsB_tiles[t % 6]
                    fxp4 = fxp.rearrange("p (c h d) -> p c h d", c=4, h=2)
                    nc.scalar.copy(fxsA[:, :, :, 0:D], fxp4[:, 0:2])
                    nc.vector.tensor_copy(fxsB[:, :, :, 0:D], fxp4[:, 2:4])
                    return fxsA, fxsB

                def emit_st(td, fxsA_d, fxsB_d):
                    swn2_d = swn_tiles[(td // 2) % 4][:, td % 2]
                    for c in range(4):
                        src = fxsA_d if c < 2 else fxsB_d
                        nc.tensor.matmul(
                            st_ps[c // 2][:, c % 2, :],
                            swn2_d[:, ts(c, 128)],
                            src[:, c % 2].rearrange("p a b -> p (a b)"),
                            start=(td == 0 and c % 2 == 0),
                            stop=(td == T - 1 and c % 2 == 1))

                fxs_hist = {}
                xt4_hist = {}

                def load_quad(q0):
                    # 8-tile input load on the gpsimd software queue (keeps
                    # both hw queues clear: sync has the transposes, scalar
                    # is compute-choked)
                    xt4 = work.tile([128, 2, 8 * TOK], BF16, tag="xt4")
                    nc.gpsimd.dma_start(xt4, xT_d[:, :, q0 * TOK:(q0 + 8) * TOK])
                    xt4_hist[q0] = xt4

                load_quad(0)
                load_quad(8)
                for t0 in range(0, T + ST_DELAY, 2):
                    if t0 % 8 == 0 and t0 + 16 < T:
                        load_quad(t0 + 16)
                    if t0 < T:
                        xt4 = xt4_hist[t0 - t0 % 8]
                        if t0 % 8 == 6:
                            del xt4_hist[t0 - 6]
                        # PE burst: mains for both tiles of the pair
                        for t in (t0, t0 + 1):
                            xt = xt4[:, :, (t % 8) * TOK:(t % 8 + 1) * TOK]
                            lg = psmm.tile([128, H * G], F32, tag="lg")
                            fxp = psmm.tile([128, INNER], F32, tag="fx")
                            for k in range(2):
                                nc.tensor.matmul(lg, xt[:, k, :], AT_sb[:, k, :],
                                                 start=(k == 0), stop=(k == 1))
                                nc.tensor.matmul(fxp, xt[:, k, :], WfxT_sb[:, k, :],
                                                 start=(k == 0), stop=(k == 1))
                            hist[t] = (lg, fxp)
                        for t in (t0, t0 + 1):
                            emit_chain(t)
                            fxs_hist[t] = emit_fxs(t)

                    # delayed slice-token accumulation (PE burst part 2)
                    for td in (t0 - ST_DELAY, t0 - ST_DELAY + 1):
                        if 0 <= td < T and td in fxs_hist:
                            emit_st(td, *fxs_hist.pop(td))

                # drain remaining delayed st matmuls
                for td in sorted(fxs_hist):
                    emit_st(td, *fxs_hist.pop(td))

                # ========== STAGE (slice attention, tiny) ==========
                # copy accumulators to SBUF (partition-aligned)
                stA = [stg_pool.tile([128, 2, 2 * (D + 1)], F32, name=f"stA{j}")
                       for j in range(2)]
                nc.vector.tensor_copy(stA[0], st_ps[0])
                nc.vector.tensor_copy(stA[1], st_ps[1])

                cc_in = dram.tile([64, H * (D + 1)], F32)
                cc_out = dram.tile([64, H * (D + 1)], F32)
                # head index h = 4j + 2k + b
                cc_in_v = cc_in.rearrange("p (j k b e) -> p j k b e", j=2, k=2, b=2)
                # head 2c   = stA[c//2][0:64,  c%2, 0:65]   (c = 2j + k)
                # head 2c+1 = stA[c//2][64:128, c%2, 65:130]
                for j in range(2):
                    nc.sync.dma_start(cc_in_v[:, j, :, 0, :],
                                      stA[j][0:64, :, 0:D + 1])
                    nc.sync.dma_start(cc_in_v[:, j, :, 1, :],
                                      stA[j][64:128, :, D + 1:2 * (D + 1)])
                nc.gpsimd.collective_compute(
                    "AllReduce", ALU.add,
                    replica_groups=[[0, 1], [2, 3], [4, 5], [6, 7]],
                    ins=[cc_in.opt()], outs=[cc_out.opt()],
                )
                stg = stg_pool.tile([64, H, D + 1], F32)
                nc.sync.dma_start(stg.rearrange("p h e -> p (h e)"), cc_out)

                snorm_e = stg_pool.tile([64, H], F32)
                nc.vector.tensor_scalar_add(snorm_e, stg[:, :, D], EPS_SLICE)
                rs = stg_pool.tile([64, H], F32)
                nc.vector.reciprocal(rs, snorm_e)
                st_sb = stg_pool.tile([64, H, D], F32)
                nc.vector.tensor_tensor(st_sb, stg[:, :, 0:D],
                                        rs[:, :, None].to_broadcast([64, H, D]),
                                        ALU.mult)
                kv = stg_pool.tile([64, D], F32)
                nc.vector.reduce_sum(kv, st_sb.rearrange("p h d -> p d h"),
                                     axis=mybir.AxisListType.X)

                # transposes of st and kv (f32, 64x64)
                stT = stg_pool.tile([64, H, D], F32)
                for h in range(H):
                    tp = psmm.tile([64, 64], F32, tag="fx")
                    nc.tensor.transpose(tp, st_sb[:, h, :], id32)
                    nc.vector.tensor_copy(stT[:, h, :], tp)
                kvT_p = psmm.tile([64, 64], F32, tag="fx")
                nc.tensor.transpose(kvT_p, kv, id32)
                kvT = stg_pool.tile([64, D], F32)
                nc.vector.tensor_copy(kvT, kvT_p)

                # q = st @ WqT (per head), k/v from kv
                q_ps = psmm.tile([64, H, D], F32, tag="lg")
                for h in range(H):
                    nc.tensor.matmul(q_ps[:, h, :], stT[:, h, :], WqT_sb,
                                     start=(h == 0), stop=(h == H - 1))
                k_ps = psmm.tile([64, D], F32, tag="fx")
                nc.tensor.matmul(k_ps, kvT, WkT_sb, start=True, stop=True)
                v_ps = psmm.tile([64, D], F32, tag="fx")
                nc.tensor.matmul(v_ps, kvT, WvT_sb, start=True, stop=True)
                v_sb = stg_pool.tile([64, D], F32)
                nc.vector.tensor_copy(v_sb, v_ps)

                def rnorm(src_ps, nh, tag):
                    # 1/sqrt(sum(src^2 over last dim)) with one Newton step
                    sq = stg_pool.tile([64, nh, D], F32, name=f"sq_{tag}")
                    nc.scalar.activation(sq, src_ps, AF.Square)
                    n2 = stg_pool.tile([64, nh], F32, name=f"n2_{tag}")
                    nc.vector.reduce_sum(n2, sq, axis=mybir.AxisListType.X)
                    r0 = stg_pool.tile([64, nh], F32, name=f"r0_{tag}")
                    nc.vector.reciprocal(r0, n2)
                    y0 = stg_pool.tile([64, nh], F32, name=f"y0_{tag}")
                    nc.scalar.activation(y0, r0, AF.Sqrt)
                    t1 = stg_pool.tile([64, nh], F32, name=f"t1_{tag}")
                    nc.vector.tensor_mul(t1, y0, y0)
                    nc.vector.tensor_mul(t1, t1, n2)
                    nc.vector.tensor_scalar(t1, t1, -0.5, 1.5, ALU.mult, ALU.add)
                    nc.vector.tensor_mul(t1, t1, y0)
                    return t1

                rq = rnorm(q_ps, H, "q")
                rk = rnorm(k_ps[:, None, :], 1, "k")

                qn = stg_pool.tile([64, H, D], F32)
                nc.vector.tensor_tensor(qn, q_ps,
                                        rq[:, :, None].to_broadcast([64, H, D]),
                                        ALU.mult)
                kn = stg_pool.tile([64, D], F32)
                nc.vector.tensor_tensor(kn, k_ps,
                                        rk[:, 0:1].to_broadcast([64, D]), ALU.mult)

                qnT = stg_pool.tile([64, H, D], F32)
                for h in range(H):
                    tp = psmm.tile([64, 64], F32, tag="fx")
                    nc.tensor.transpose(tp, qn[:, h, :], id32)
                    nc.vector.tensor_copy(qnT[:, h, :], tp)
                knT_p = psmm.tile([64, 64], F32, tag="fx")
                nc.tensor.transpose(knT_p, kn, id32)
                knT = stg_pool.tile([64, D], F32)
                nc.vector.tensor_copy(knT, knT_p)

                # attention logits both orientations, exp, denominators
                L_ps = psmm.tile([64, H, G], F32, tag="lg")
                for h in range(H):
                    nc.tensor.matmul(L_ps[:, h, :], qnT[:, h, :], knT,
                                     start=(h == 0), stop=(h == H - 1))
                e_sb = stg_pool.tile([64, H, G], F32)
                nc.scalar.activation(e_sb, L_ps, AF.Exp, scale=attn_scale)
                aden = stg_pool.tile([64, H], F32)
                nc.vector.reduce_sum(aden, e_sb, axis=mybir.AxisListType.X)
                ra = stg_pool.tile([64, H], F32)
                nc.vector.reciprocal(ra, aden)

                LT_ps = psmm.tile([64, H, G], F32, tag="fx")
                for h in range(H):
                    nc.tensor.matmul(LT_ps[:, h, :], knT, qnT[:, h, :],
                                     start=(h == 0), stop=(h == H - 1))
                eT_sb = stg_pool.tile([64, H, G], F32)
                nc.scalar.activation(eT_sb, LT_ps, AF.Exp, scale=attn_scale)

                av_ps = psmm.tile([64, H, D], F32, tag="lg")
                for h in range(H):
                    nc.tensor.matmul(av_ps[:, h, :], eT_sb[:, h, :], v_sb,
                                     start=(h == 0), stop=(h == H - 1))

                os_sb = stg_pool.tile([64, H, D], F32)
                nc.vector.tensor_tensor(os_sb, av_ps,
                                        ra[:, :, None].to_broadcast([64, H, D]),
                                        ALU.mult)
                rst = stg_pool.tile([64, H, D], F32)
                nc.vector.tensor_scalar_mul(rst, st_sb, res_scale)
                nc.vector.tensor_add(os_sb, os_sb, rst)

                osT = stg_pool.tile([64, H, D], BF16)
                for h in range(H):
                    tp = psmm.tile([64, 64], F32, tag="fx")
                    nc.tensor.transpose(tp, os_sb[:, h, :], id32)
                    nc.vector.tensor_copy(osT[:, h, :], tp)

                for j in range(4):
                    C_ps = psmm.tile([128, DIM], F32, tag="lg")
                    for par in range(2):
                        h = 2 * j + par
                        nc.tensor.matmul(C_ps[64 * par:64 * par + 64, :],
                                         osT[:, h, :], WoT_sb[:, h, :],
                                         start=True, stop=True)
                    nc.vector.tensor_copy(C_sb[:, j, :], C_ps)

            # ================= PASS 2 =================
            # transposed-output formulation: for each 4-tile group, compute
            # outT[f, tok] = sum_c C[c-chunk, f].T @ swT[c-chunk, 4 tiles of
            # tok] with a 512-wide moving operand — half the matmuls of the
            # [tok, f] orientation. The host transposes back.
            with tc.tile_pool(name="ps2", bufs=3, space="PSUM") as ps2:
                ob = None
                for g in range(T // 4):
                    t0 = 4 * g
                    opa = ps2.tile([128, 4 * TOK], F32, tag="p2a")
                    opb = ps2.tile([128, 4 * TOK], F32, tag="p2b")
                    for c in range(4):
                        rhs = swT_store[:, t0:t0 + 4, c, :]
                        nc.tensor.matmul(opa, C_sb[:, c, 0:128], rhs,
                                         start=(c == 0), stop=(c == 3))
                        nc.tensor.matmul(opb, C_sb[:, c, 128:256], rhs,
                                         start=(c == 0), stop=(c == 3))
                    if g % 2 == 0:
                        ob = obuf.tile([128, 2, 2, 4 * TOK], BF16, tag="ob")
                        nc.vector.tensor_copy(ob[:, 0, 0], opa)
                        nc.vector.tensor_copy(ob[:, 1, 0], opb)
                    else:
                        nc.vector.tensor_copy(ob[:, 0, 1], opa)
                        nc.vector.tensor_copy(ob[:, 1, 1], opb)
                        # alternate output DMAs across both hw queues; the
                        # PSUM-freeing copies all live on vector so neither
                        # queue's transfers can stall them
                        eng = nc.sync if (g // 2) % 2 == 0 else nc.scalar
                        eng.dma_start(
                            outT_v[:, :, (t0 - 4) * TOK:(t0 + 4) * TOK],
                            ob.rearrange("p fc g j -> p fc (g j)"))

    nc.finalize()
    return nc


def kernel(x, Wfx, bfx, Wx, bx, Wslice, bslice, temp, Wq, Wk, Wv,
           res_scale, attn_scale, Wout, bout):
    x = np.asarray(x, dtype=np.float32)
    Wfx = np.asarray(Wfx, np.float32); bfx = np.asarray(bfx, np.float32)
    Wx = np.asarray(Wx, np.float32); bx = np.asarray(bx, np.float32)
    Wslice = np.asarray(Wslice, np.float32); bslice = np.asarray(bslice, np.float32)
    temp = np.asarray(temp, np.float32).reshape(H)
    Wq = np.asarray(Wq, np.float32); Wk = np.asarray(Wk, np.float32)
    Wv = np.asarray(Wv, np.float32)
    res_scale_f = float(np.asarray(res_scale, np.float32))
    attn = np.asarray(attn_scale, np.float32).reshape(H)
    Wout = np.asarray(Wout, np.float32); bout = np.asarray(bout, np.float32)

    assert np.all(np.abs(bfx) == 0) and np.all(np.abs(bx) == 0) \
        and np.all(np.abs(bslice) == 0), "nonzero projection biases unsupported"
    assert np.ptp(attn) == 0, "non-uniform attn_scale unsupported"
    attn_f = float(attn[0])

    # folded logits weight: logits[:, h*G+g] = x @ ((Wslice @ Wx_h)/temp_h).T
    A = np.concatenate(
        [(Wslice @ Wx[h * D:(h + 1) * D, :]) / temp[h] for h in range(H)], axis=0)
    BFNP = ml_dtypes.bfloat16

    def chunk_major(w):  # [256, cols] -> [128, 2, cols]
        return np.ascontiguousarray(w.reshape(2, 128, -1).transpose(1, 0, 2))

    AT = chunk_major(A.T.astype(BFNP))                    # [128, 2, 512]
    WfxT = chunk_major(Wfx.T.astype(BFNP))                # [128, 2, 512]
    WoT = np.ascontiguousarray(Wout.T).astype(ml_dtypes.bfloat16)  # [512, 256]
    WqT = np.ascontiguousarray(Wq.T)
    WkT = np.ascontiguousarray(Wk.T) / H
    WvT = np.ascontiguousarray(Wv.T) / H
    id32 = np.eye(64, dtype=np.float32)

    key = (attn_f, res_scale_f)
    if key not in _CACHE:
        _CACHE[key] = _build(attn_f, res_scale_f)
    nc = _CACHE[key]

    in_maps = []
    for c in range(NCORES):
        b, half = c // 2, c % 2
        xs = x[b, half * NLOC:(half + 1) * NLOC, :]       # [16384, 256]
        xT = chunk_major(np.ascontiguousarray(xs.T).astype(BFNP))  # [128,2,NLOC]
        in_maps.append(dict(xT=xT, AT=AT, WfxT=WfxT, id32=id32,
                            WqT=WqT, WkT=WkT, WvT=WvT, WoT=WoT))

    global _LAST_IN_MAPS
    _LAST_IN_MAPS = in_maps
    res = bass_utils.run_bass_kernel_spmd(nc, in_maps, core_ids=list(range(NCORES)))

    out = np.empty((B, N, DIM), np.float32)
    for c in range(NCORES):
        b, half = c // 2, c % 2
        oT = res.results[c]["outT"].reshape(DIM, NLOC)
        out[b, half * NLOC:(half + 1) * NLOC, :] = \
            np.ascontiguousarray(oT.T).astype(np.float32)
    if np.any(bout):
        out += bout
    return out


# revision 36
# speedup vs baseline: 1.1412x; 1.0236x over previous
"""Trainium2 Bass kernel for Physics-Attention over an irregular mesh.

Contract: kernel(**inputs) takes the FULL inputs from setup_inputs() and
returns the FULL [4, 32768, 256] f32 output, distributing across 8 cores
internally (one (batch, half-of-N) shard per core, pairwise AllReduce on the
slice-token pooling reductions).

Structure (single core):
  pass 1 (per 128-token tile): logits/fx matmuls (bf16, chunked K=256),
    exp -> per-head denom -> reciprocal -> normalized routing weights swn;
    slice-token accumulation via pair-merged fp32-accumulating matmuls,
    software-pipelined ST_DELAY tiles behind the mains so the PE stays in
    long bursts (high p-state); swn is transposed for pass 2 by the DMA
    XBAR (sync queue), input loads ride the gpsimd software queue.
  stage: pairwise AllReduce of slice-token partials, tiny slice attention.
  pass 2: out = swT @ C in 8-tile supertiles, psum-bank-rotated matmuls,
    bf16 output written via batched DMA (host upcasts to f32).
"""

import sys

sys.path.insert(0, "/opt/trn_rl_repo")

import numpy as np
import ml_dtypes

import concourse.bass as bass
import concourse.mybir as mybir
import concourse.tile as tile
from concourse import bacc, bass_utils
from concourse.bass import ts

F32 = mybir.dt.float32
BF16 = mybir.dt.bfloat16
AF = mybir.ActivationFunctionType
ALU = mybir.AluOpType

B, N, DIM = 4, 32768, 256
H, D, G = 8, 64, 64
INNER = H * D  # 512
NCORES = 8
NLOC = N // 2          # 16384 tokens per core
TOK = 128              # tokens per tile
T = NLOC // TOK        # 128 tiles
EPS_SLICE = 1e-5
ST_DELAY = 4           # software-pipeline delay of the st matmuls (tiles)
SUP = 8                # pass-2 supertile (tiles per output DMA)

_CACHE = {}


def _build(attn_scale: float, res_scale: float, debug: bool = False):
    """Build the single-core SPMD program (identical on all 8 cores)."""
    nc = bacc.Bacc("TRN2", target_bir_lowering=False, debug=False,
                   enable_asserts=False, num_devices=NCORES)

    xT_d = nc.dram_tensor("xT", [128, 2, NLOC], BF16, kind="ExternalInput").ap()
    AT_d = nc.dram_tensor("AT", [128, 2, INNER], BF16, kind="ExternalInput").ap()
    WfxT_d = nc.dram_tensor("WfxT", [128, 2, INNER], BF16, kind="ExternalInput").ap()
    id32_d = nc.dram_tensor("id32", [64, 64], F32, kind="ExternalInput").ap()
    WqT_d = nc.dram_tensor("WqT", [D, D], F32, kind="ExternalInput").ap()
    WkT_d = nc.dram_tensor("WkT", [D, D], F32, kind="ExternalInput").ap()
    WvT_d = nc.dram_tensor("WvT", [D, D], F32, kind="ExternalInput").ap()
    WoT_d = nc.dram_tensor("WoT", [INNER, DIM], BF16, kind="ExternalInput").ap()
    outT_d = nc.dram_tensor("outT", [2, 128, NLOC], BF16, kind="ExternalOutput").ap()

    WoT_v = WoT_d.rearrange("(h d) f -> d h f", d=64)   # [64, 8, 256]
    # transposed output view: [p, fc, n] for one DMA per 4-tile group
    outT_v = outT_d.rearrange("fc p n -> p fc n")

    with tile.TileContext(nc) as tc:
        with (
            tc.tile_pool(name="consts", bufs=1) as consts,
            tc.tile_pool(name="store", bufs=1) as store,
            tc.tile_pool(name="work", bufs=3) as work,
            tc.tile_pool(name="uswp", bufs=8) as uswp,
            tc.tile_pool(name="small", bufs=6) as small,
            tc.tile_pool(name="stage", bufs=1) as stg_pool,
            tc.tile_pool(name="obuf", bufs=2) as obuf,
            tc.tile_pool(name="dram", bufs=1, space="DRAM") as dram,
        ):
            # ---- resident constants ----
            AT_sb = consts.tile([128, 2, INNER], BF16)
            nc.sync.dma_start(AT_sb, AT_d)
            WfxT_sb = consts.tile([128, 2, INNER], BF16)
            nc.sync.dma_start(WfxT_sb, WfxT_d)
            id32 = consts.tile([64, 64], F32)
            nc.sync.dma_start(id32, id32_d)
            WqT_sb = consts.tile([64, 64], F32)
            nc.sync.dma_start(WqT_sb, WqT_d)
            WkT_sb = consts.tile([64, 64], F32)
            nc.sync.dma_start(WkT_sb, WkT_d)
            WvT_sb = consts.tile([64, 64], F32)
            nc.sync.dma_start(WvT_sb, WvT_d)
            WoT_sb = consts.tile([64, H, DIM], BF16)
            nc.sync.dma_start(WoT_sb, WoT_v)

            # transposed routing weights, written by DMA transpose:
            # swT_store[p, t, c, j] = swn_t[j, c*128 + p]
            swT_store = store.tile([128, T, 4, TOK], BF16)

            # persistent swn pair tiles (4-deep manual rotation): pair P holds
            # tiles 2P, 2P+1; DMA-transposed together after the odd tile
            swn_tiles = [consts.tile([128, 2, H * G], BF16, name=f"swn{i}")
                         for i in range(4)]

            # persistent fxs tiles (6-deep manual rotation), ones columns
            # preset once: fxs*[p, c, half, 0:64] = fx data, [.., 64] = 1.
            # Split into a scalar-written set (pairs 0-1) and a vector-written
            # set (pairs 2-3) so the two engines share no tile (avoids false
            # WAW lockstep).
            fxsA_tiles = [consts.tile([128, 2, 2, D + 1], BF16, name=f"fxsA{i}")
                          for i in range(6)]
            fxsB_tiles = [consts.tile([128, 2, 2, D + 1], BF16, name=f"fxsB{i}")
                          for i in range(6)]
            for i in range(6):
                nc.gpsimd.memset(fxsA_tiles[i][:, :, :, D], 1.0)
                nc.gpsimd.memset(fxsB_tiles[i][:, :, :, D], 1.0)

            C_sb = stg_pool.tile([128, 4, DIM], BF16)

            with (
                tc.tile_pool(name="psmm", bufs=3, space="PSUM") as psmm,
                tc.tile_pool(name="psacc", bufs=1, space="PSUM") as psacc,
            ):
                # slice-token accumulators: pair c = heads (2c, 2c+1) lives in
                # st_ps[c//2][:, c%2, :]; valid regions: head 2c ->
                # [0:64, 0:65], head 2c+1 -> [64:128, 65:130].
                st_ps = [psacc.tile([128, 2, 2 * (D + 1)], F32, name=f"st_ps{j}")
                         for j in range(2)]

                # ================= PASS 1 =================
                # processed in tile PAIRS: PE does mains for both tiles
                # back-to-back, then the delayed st matmuls for an earlier
                # pair — long PE bursts keep the tensor engine at high pstate.
                # The elementwise chain (exp/reduce/recip/mult) is fused at
                # pair granularity to amortize fixed costs; fxs casts are
                # emitted after the chain so they can't block the next exp.
                hist = {}  # tile t -> (lg, fxp) psum handles

                def emit_chain(t):
                    lg, fxp = hist[t]
                    usw = uswp.tile([128, H, G], BF16, tag="usw")
                    nc.scalar.activation(usw, lg.rearrange("p (h g) -> p h g", h=H),
                                         AF.Exp)
                    den = small.tile([128, H], BF16, tag="den")
                    rden = small.tile([128, H], BF16, tag="rden")
                    with nc.allow_low_precision(reason="softmax denom in bf16"):
                        nc.vector.reduce_sum(den, usw, axis=mybir.AxisListType.X)
                        nc.vector.reciprocal(rden, den)
                    swn_pair = swn_tiles[(t // 2) % 4]
                    swn = swn_pair[:, t % 2].rearrange("p (h g) -> p h g", h=H)
                    nc.gpsimd.tensor_tensor(
                        swn, usw, rden[:, :, None].to_broadcast([128, H, G]),
                        ALU.mult)
                    if t % 2 == 1:
                        # transposed copy for pass 2, on the DMA engines
                        nc.sync.dma_start(
                            swT_store[:, t - 1:t + 1].rearrange(
                                "p a b j -> p (a b) j"),
                            swn_pair.rearrange("p a f -> p (a f)"),
                            transpose=True)

                def emit_fxs(t):
                    _, fxp = hist.pop(t)
                    fxsA = fxsA_tiles[t % 6]
                    fxsB = fxsB_tiles[t % 6]
                    fxp4 = fxp.rearrange("p (c h d) -> p c h d", c=4, h=2)
                    nc.scalar.copy(fxsA[:, :, :, 0:D], fxp4[:, 0:2])
                    nc.vector.tensor_copy(fxsB[:, :, :, 0:D], fxp4[:, 2:4])
                    return fxsA, fxsB

                def emit_st(td, fxsA_d, fxsB_d):
                    swn2_d = swn_tiles[(td // 2) % 4][:, td % 2]
                    for c in range(4):
                        src = fxsA_d if c < 2 else fxsB_d
                        nc.tensor.matmul(
                            st_ps[c // 2][:, c % 2, :],
                            swn2_d[:, ts(c, 128)],
                            src[:, c % 2].rearrange("p a b -> p (a b)"),
                            start=(td == 0 and c % 2 == 0),
                            stop=(td == T - 1 and c % 2 == 1))

                fxs_hist = {}
                xt4_hist = {}

                def load_quad(q0):
                    # 8-tile input load on the gpsimd software queue (keeps
                    # both hw queues clear: sync has the transposes, scalar
                    # is compute-choked)
                    xt4 = work.tile([128, 2, 8 * TOK], BF16, tag="xt4")
                    nc.gpsimd.dma_start(xt4, xT_d[:, :, q0 * TOK:(q0 + 8) * TOK])
                    xt4_hist[q0] = xt4

                load_quad(0)
                load_quad(8)
                for t0 in range(0, T + ST_DELAY, 2):
                    if t0 % 8 == 0 and t0 + 16 < T:
                        load_quad(t0 + 16)
                    if t0 < T:
                        xt4 = xt4_hist[t0 - t0 % 8]
                        if t0 % 8 == 6:
                            del xt4_hist[t0 - 6]
                        # PE burst: mains for both tiles of the pair
                        for t in (t0, t0 + 1):
                            xt = xt4[:, :, (t % 8) * TOK:(t % 8 + 1) * TOK]
                            lg = psmm.tile([128, H * G], F32, tag="lg")
                            fxp = psmm.tile([128, INNER], F32, tag="fx")
                            for k in range(2):
                                nc.tensor.matmul(lg, xt[:, k, :], AT_sb[:, k, :],
                                                 start=(k == 0), stop=(k == 1))
                                nc.tensor.matmul(fxp, xt[:, k, :], WfxT_sb[:, k, :],
                                                 start=(k == 0), stop=(k == 1))
                            hist[t] = (lg, fxp)
                        for t in (t0, t0 + 1):
                            emit_chain(t)
                            fxs_hist[t] = emit_fxs(t)

                    # delayed slice-token accumulation (PE burst part 2)
                    for td in (t0 - ST_DELAY, t0 - ST_DELAY + 1):
                        if 0 <= td < T and td in fxs_hist:
                            emit_st(td, *fxs_hist.pop(td))

                # drain remaining delayed st matmuls
                for td in sorted(fxs_hist):
                    emit_st(td, *fxs_hist.pop(td))

                # ========== STAGE (slice attention, tiny) ==========
                # copy accumulators to SBUF (partition-aligned)
                stA = [stg_pool.tile([128, 2, 2 * (D + 1)], F32, name=f"stA{j}")
                       for j in range(2)]
                nc.vector.tensor_copy(stA[0], st_ps[0])
                nc.vector.tensor_copy(stA[1], st_ps[1])

                cc_in = dram.tile([64, H * (D + 1)], F32)
                cc_out = dram.tile([64, H * (D + 1)], F32)
                # head index h = 4j + 2k + b
                cc_in_v = cc_in.rearrange("p (j k b e) -> p j k b e", j=2, k=2, b=2)
                # head 2c   = stA[c//2][0:64,  c%2, 0:65]   (c = 2j + k)
                # head 2c+1 = stA[c//2][64:128, c%2, 65:130]
                for j in range(2):
                    nc.sync.dma_start(cc_in_v[:, j, :, 0, :],
                                      stA[j][0:64, :, 0:D + 1])
                    nc.sync.dma_start(cc_in_v[:, j, :, 1, :],
                                      stA[j][64:128, :, D + 1:2 * (D + 1)])
                nc.gpsimd.collective_compute(
                    "AllReduce", ALU.add,
                    replica_groups=[[0, 1], [2, 3], [4, 5], [6, 7]],
                    ins=[cc_in.opt()], outs=[cc_out.opt()],
                )
                stg = stg_pool.tile([64, H, D + 1], F32)
                nc.sync.dma_start(stg.rearrange("p h e -> p (h e)"), cc_out)

                snorm_e = stg_pool.tile([64, H], F32)
                nc.vector.tensor_scalar_add(snorm_e, stg[:, :, D], EPS_SLICE)
                rs = stg_pool.tile([64, H], F32)
                nc.vector.reciprocal(rs, snorm_e)
                st_sb = stg_pool.tile([64, H, D], F32)
                nc.vector.tensor_tensor(st_sb, stg[:, :, 0:D],
                                        rs[:, :, None].to_broadcast([64, H, D]),
                                        ALU.mult)
                kv = stg_pool.tile([64, D], F32)
                nc.vector.reduce_sum(kv, st_sb.rearrange("p h d -> p d h"),
                                     axis=mybir.AxisListType.X)

                # transposes of st and kv (f32, 64x64)
                stT = stg_pool.tile([64, H, D], F32)
                for h in range(H):
                    tp = psmm.tile([64, 64], F32, tag="fx")
                    nc.tensor.transpose(tp, st_sb[:, h, :], id32)
                    nc.vector.tensor_copy(stT[:, h, :], tp)
                kvT_p = psmm.tile([64, 64], F32, tag="fx")
                nc.tensor.transpose(kvT_p, kv, id32)
                kvT = stg_pool.tile([64, D], F32)
                nc.vector.tensor_copy(kvT, kvT_p)

                # q = st @ WqT (per head), k/v from kv
                q_ps = psmm.tile([64, H, D], F32, tag="lg")
                for h in range(H):
                    nc.tensor.matmul(q_ps[:, h, :], stT[:, h, :], WqT_sb,
                                     start=(h == 0), stop=(h == H - 1))
                k_ps = psmm.tile([64, D], F32, tag="fx")
                nc.tensor.matmul(k_ps, kvT, WkT_sb, start=True, stop=True)
                v_ps = psmm.tile([64, D], F32, tag="fx")
                nc.tensor.matmul(v_ps, kvT, WvT_sb, start=True, stop=True)
                v_sb = stg_pool.tile([64, D], F32)
                nc.vector.tensor_copy(v_sb, v_ps)

                def rnorm(src_ps, nh, tag):
                    # 1/sqrt(sum(src^2 over last dim)) with one Newton step
                    sq = stg_pool.tile([64, nh, D], F32, name=f"sq_{tag}")
                    nc.scalar.activation(sq, src_ps, AF.Square)
                    n2 = stg_pool.tile([64, nh], F32, name=f"n2_{tag}")
                    nc.vector.reduce_sum(n2, sq, axis=mybir.AxisListType.X)
                    r0 = stg_pool.tile([64, nh], F32, name=f"r0_{tag}")
                    nc.vector.reciprocal(r0, n2)
                    y0 = stg_pool.tile([64, nh], F32, name=f"y0_{tag}")
                    nc.scalar.activation(y0, r0, AF.Sqrt)
                    t1 = stg_pool.tile([64, nh], F32, name=f"t1_{tag}")
                    nc.vector.tensor_mul(t1, y0, y0)
                    nc.vector.tensor_mul(t1, t1, n2)
                    nc.vector.tensor_scalar(t1, t1, -0.5, 1.5, ALU.mult, ALU.add)
                    nc.vector.tensor_mul(t1, t1, y0)
                    return t1

                rq = rnorm(q_ps, H, "q")
                rk = rnorm(k_ps[:, None, :], 1, "k")

                qn = stg_pool.tile([64, H, D], F32)
                nc.vector.tensor_tensor(qn, q_ps,
                                        rq[:, :, None].to_broadcast([64, H, D]),
                                        ALU.mult)
                kn = stg_pool.tile([64, D], F32)
                nc.vector.tensor_tensor(kn, k_ps,
                                        rk[:, 0:1].to_broadcast([64, D]), ALU.mult)

                qnT = stg_pool.tile([64, H, D], F32)
                for h in range(H):
                    tp = psmm.tile([64, 64], F32, tag="fx")
                    nc.tensor.transpose(tp, qn[:, h, :], id32)
                    nc.vector.tensor_copy(qnT[:, h, :], tp)
                knT_p = psmm.tile([64, 64], F32, tag="fx")
                nc.tensor.transpose(knT_p, kn, id32)
                knT = stg_pool.tile([64, D], F32)
                nc.vector.tensor_copy(knT, knT_p)

                # attention logits both orientations, exp, denominators
                L_ps = psmm.tile([64, H, G], F32, tag="lg")
                for h in range(H):
                    nc.tensor.matmul(L_ps[:, h, :], qnT[:, h, :], knT,
                                     start=(h == 0), stop=(h == H - 1))
                e_sb = stg_pool.tile([64, H, G], F32)
                nc.scalar.activation(e_sb, L_ps, AF.Exp, scale=attn_scale)
                aden = stg_pool.tile([64, H], F32)
                nc.vector.reduce_sum(aden, e_sb, axis=mybir.AxisListType.X)
                ra = stg_pool.tile([64, H], F32)
                nc.vector.reciprocal(ra, aden)

                LT_ps = psmm.tile([64, H, G], F32, tag="fx")
                for h in range(H):
                    nc.tensor.matmul(LT_ps[:, h, :], knT, qnT[:, h, :],
                                     start=(h == 0), stop=(h == H - 1))
                eT_sb = stg_pool.tile([64, H, G], F32)
                nc.scalar.activation(eT_sb, LT_ps, AF.Exp, scale=attn_scale)

                av_ps = psmm.tile([64, H, D], F32, tag="lg")
                for h in range(H):
                    nc.tensor.matmul(av_ps[:, h, :], eT_sb[:, h, :], v_sb,
                                     start=(h == 0), stop=(h == H - 1))

                os_sb = stg_pool.tile([64, H, D], F32)
                nc.vector.tensor_tensor(os_sb, av_ps,
                                        ra[:, :, None].to_broadcast([64, H, D]),
                                        ALU.mult)
                rst = stg_pool.tile([64, H, D], F32)
                nc.vector.tensor_scalar_mul(rst, st_sb, res_scale)
                nc.vector.tensor_add(os_sb, os_sb, rst)

                osT = stg_pool.tile([64, H, D], BF16)
                for h in range(H):
                    tp = psmm.tile([64, 64], F32, tag="fx")
                    nc.tensor.transpose(tp, os_sb[:, h, :], id32)
                    nc.vector.tensor_copy(osT[:, h, :], tp)

                for j in range(4):
                    C_ps = psmm.tile([128, DIM], F32, tag="lg")
                    for par in range(2):
                        h = 2 * j + par
                        nc.tensor.matmul(C_ps[64 * par:64 * par + 64, :],
                                         osT[:, h, :], WoT_sb[:, h, :],
                                         start=True, stop=True)
                    nc.vector.tensor_copy(C_sb[:, j, :], C_ps)

            # ================= PASS 2 =================
            # transposed-output formulation: for each 4-tile group, compute
            # outT[f, tok] = sum_c C[c-chunk, f].T @ swT[c-chunk, 4 tiles of
            # tok] with a 512-wide moving operand — half the matmuls of the
            # [tok, f] orientation. The host transposes back.
            with tc.tile_pool(name="ps2", bufs=3, space="PSUM") as ps2:
                ob = None
                for g in range(T // 4):
                    t0 = 4 * g
                    opa = ps2.tile([128, 4 * TOK], F32, tag="p2a")
                    opb = ps2.tile([128, 4 * TOK], F32, tag="p2b")
                    for c in range(4):
                        rhs = swT_store[:, t0:t0 + 4, c, :]
                        nc.tensor.matmul(opa, C_sb[:, c, 0:128], rhs,
                                         start=(c == 0), stop=(c == 3))
                        nc.tensor.matmul(opb, C_sb[:, c, 128:256], rhs,
                                         start=(c == 0), stop=(c == 3))
                    # copies: mostly vector, every 3rd group on scalar so
                    # neither engine is the wall (scalar also issues half the
                    # output DMAs; vector issues none)
                    cp = nc.scalar.copy if g % 3 == 2 else nc.vector.tensor_copy
                    if g % 2 == 0:
                        ob = obuf.tile([128, 2, 2, 4 * TOK], BF16, tag="ob")
                        cp(ob[:, 0, 0], opa)
                        cp(ob[:, 1, 0], opb)
                    else:
                        cp(ob[:, 0, 1], opa)
                        cp(ob[:, 1, 1], opb)
                        # alternate output DMAs across both hw queues; the
                        # PSUM-freeing copies all live on vector so neither
                        # queue's transfers can stall them
                        eng = nc.sync if (g // 2) % 2 == 0 else nc.scalar
                        eng.dma_start(
                            outT_v[:, :, (t0 - 4) * TOK:(t0 + 4) * TOK],
                            ob.rearrange("p fc g j -> p fc (g j)"))

    nc.finalize()
    return nc


def kernel(x, Wfx, bfx, Wx, bx, Wslice, bslice, temp, Wq, Wk, Wv,
           res_scale, attn_scale, Wout, bout):
    x = np.asarray(x, dtype=np.float32)
    Wfx = np.asarray(Wfx, np.float32); bfx = np.asarray(bfx, np.float32)
    Wx = np.asarray(Wx, np.float32); bx = np.asarray(bx, np.float32)
    Wslice = np.asarray(Wslice, np.float32); bslice = np.asarray(bslice, np.float32)
    temp = np.asarray(temp, np.float32).reshape(H)
    Wq = np.asarray(Wq, np.float32); Wk = np.asarray(Wk, np.float32)
    Wv = np.asarray(Wv, np.float32)
    res_scale_f = float(np.asarray(res_scale, np.float32))
    attn = np.asarray(attn_scale, np.float32).reshape(H)
    Wout = np.asarray(Wout, np.float32); bout = np.asarray(bout, np.float32)

    assert np.all(np.abs(bfx) == 0) and np.all(np.abs(bx) == 0) \
        and np.all(np.abs(bslice) == 0), "nonzero projection biases unsupported"
    assert np.ptp(attn) == 0, "non-uniform attn_scale unsupported"
    attn_f = float(attn[0])

    # folded logits weight: logits[:, h*G+g] = x @ ((Wslice @ Wx_h)/temp_h).T
    A = np.concatenate(
        [(Wslice @ Wx[h * D:(h + 1) * D, :]) / temp[h] for h in range(H)], axis=0)
    BFNP = ml_dtypes.bfloat16

    def chunk_major(w):  # [256, cols] -> [128, 2, cols]
        return np.ascontiguousarray(w.reshape(2, 128, -1).transpose(1, 0, 2))

    AT = chunk_major(A.T.astype(BFNP))                    # [128, 2, 512]
    WfxT = chunk_major(Wfx.T.astype(BFNP))                # [128, 2, 512]
    WoT = np.ascontiguousarray(Wout.T).astype(ml_dtypes.bfloat16)  # [512, 256]
    WqT = np.ascontiguousarray(Wq.T)
    WkT = np.ascontiguousarray(Wk.T) / H
    WvT = np.ascontiguousarray(Wv.T) / H
    id32 = np.eye(64, dtype=np.float32)

    key = (attn_f, res_scale_f)
    if key not in _CACHE:
        _CACHE[key] = _build(attn_f, res_scale_f)
    nc = _CACHE[key]

    in_maps = []
    for c in range(NCORES):
        b, half = c // 2, c % 2
        xs = x[b, half * NLOC:(half + 1) * NLOC, :]       # [16384, 256]
        xT = chunk_major(np.ascontiguousarray(xs.T).astype(BFNP))  # [128,2,NLOC]
        in_maps.append(dict(xT=xT, AT=AT, WfxT=WfxT, id32=id32,
                            WqT=WqT, WkT=WkT, WvT=WvT, WoT=WoT))

    global _LAST_IN_MAPS
    _LAST_IN_MAPS = in_maps
    res = bass_utils.run_bass_kernel_spmd(nc, in_maps, core_ids=list(range(NCORES)))

    out = np.empty((B, N, DIM), np.float32)
    for c in range(NCORES):
        b, half = c // 2, c % 2
        oT = res.results[c]["outT"].reshape(DIM, NLOC)
        out[b, half * NLOC:(half + 1) * NLOC, :] = \
            np.ascontiguousarray(oT.T).astype(np.float32)
    if np.any(bout):
        out += bout
    return out
